# revision 11
# baseline (speedup 1.0000x reference)
"""Trainium2 Bass kernel for nn_ChannelNonlinearSpectralBlock.

Math
----
Per pixel column x (C=256 channels), the reference computes
    u  = g(||x||) * x                      (log map, per-pixel scalar gate)
    u1 = f1(||u||) * u                     (Fourier gate 1)
    v0 = irfft(rfft(u1) * Hf)              (fixed linear map: circulant Wc)
    v1 = f2(||v0||) * v0                   (Fourier gate 2)
    y  = t(||v1||) * v1                    (exp map)
    out = alpha*y + beta*x
All per-pixel scalars commute through the linear map Wc, so
    out = A * (Wc @ x) + beta * x
with A = alpha * P1(r0) * Q2(r2), where r0 = ||x||, r2 = |P1| * q,
q^2 = r0^2 - d0*S0^2 - d1*S1^2 (Parseval; S0 = sum_c x_c, S1 = alternating
sum).  P1, Q2 are smooth 1-D functions of r fitted host-side as Chebyshev
approximants and evaluated on-device in factored quadratic form
    P(r) = aN * prod_j (r^2 + p_j r + q_j)
using fused scalar_tensor_tensor ops (2 DVE ops per quadratic factor).
Fit domains are exact data ranges computed host-side from x.

Per chunk of 2048 pixels, channel blocks are folded (xs = x0+x1,
sqs = x0^2+x1^2) before the ones-matmul reduce, halving PE streams; the
three per-pixel stats land in one PSUM tile (partitions 0,1 and 32 via
matmul tile_position) so one Scalar copy drains them, then tiny
SBUF->SBUF DMAs pack the [128, 64] chain layout.

Sharding: pure data parallel over pixels; core k takes images [4k, 4k+4).
"""

import numpy as np

import concourse.bass as bass
import concourse.bacc as bacc
from concourse import library_config
import concourse.mybir as mybir
from concourse.tile import TileContext

F32 = mybir.dt.float32
F32R = mybir.dt.float32r

# Problem shape (hardcoded per contract)
B, C, H, W = 32, 256, 64, 64
HWPIX = H * W  # 4096
NCORES = 8
B_CORE = B // NCORES  # 4 images per core
NPIX = B_CORE * HWPIX  # 16384 pixels per core
HALF = NPIX // 2  # 8192
N_HALF_SUB = HALF // 512  # 16 subtiles of 512 px per half
CHUNK = 2048  # phase-1 chunk (pixels)
N_CHUNK = NPIX // CHUNK  # 8

C_CURV = 0.001
L = 10
N_HARM = 16
EPS = 1e-6

DEG = 16  # Chebyshev degree; even => DEG//2 quadratic factors
NQ = DEG // 2

# cvec layout
ID_P1P = 0
ID_P1Q = ID_P1P + NQ
ID_A1N = ID_P1Q + NQ
ID_Q2P = ID_A1N + 1
ID_Q2Q = ID_Q2P + NQ
ID_A2N = ID_Q2Q + NQ
ID_ND0 = ID_A2N + 1
ID_ND1 = ID_ND0 + 1
ID_R0LO = ID_ND1 + 1
ID_R0HI = ID_R0LO + 1
ID_R2LO = ID_R0HI + 1
ID_R2HI = ID_R2LO + 1
ID_BETA = ID_R2HI + 1
CV = 64
assert ID_BETA < CV


def build_program(beta_nonzero: bool) -> bass.Bass:
    nc = bacc.Bacc(None, target_bir_lowering=False)
    x = nc.declare_dram_parameter("x", [B_CORE, C, HWPIX], F32, isOutput=False)
    wmat = nc.declare_dram_parameter("wmat", [2, 2, 128, 128], F32, isOutput=False)
    onesv = nc.declare_dram_parameter("onesv", [128, 32], F32, isOutput=False)
    cvec = nc.declare_dram_parameter("cvec", [1, CV], F32, isOutput=False)
    out = nc.declare_dram_parameter("out", [B_CORE, C, HWPIX], F32, isOutput=True)

    # channel-block views of DRAM x / out: [cb, 128, b, hw]
    xv = x.rearrange("b (cb p) hw -> cb p b hw", cb=2)
    ov = out.rearrange("b (cb p) hw -> cb p b hw", cb=2)

    AT = mybir.AluOpType
    ACT = mybir.ActivationFunctionType

    with TileContext(nc) as tc:
        with (
            tc.tile_pool(name="const", bufs=1) as const,
            tc.tile_pool(name="xres", bufs=1) as xres,
            tc.tile_pool(name="sq", bufs=1) as sqp,
            tc.tile_pool(name="xsf", bufs=2) as xsp,
            tc.tile_pool(name="stg", bufs=3) as stgp,
            tc.tile_pool(name="chain", bufs=1) as chp,
            tc.tile_pool(name="astage", bufs=4) as asp,
            tc.tile_pool(name="abp", bufs=3) as abp,
            tc.tile_pool(name="outp", bufs=3) as outp,
            tc.tile_pool(name="stps", bufs=2, space="PSUM") as stps,
            tc.tile_pool(name="wps", bufs=2, space="PSUM") as wps,
        ):
            nc.gpsimd.load_library(library_config.mlp)

            # ---- constants ----
            wm = [[None, None], [None, None]]
            for kb in range(2):
                for mb in range(2):
                    t = const.tile(
                        [128, 128], F32R, name=f"wm{kb}{mb}", tag=f"wm{kb}{mb}"
                    )
                    nc.sync.dma_start(out=t, in_=wmat[kb, mb].bitcast(F32R))
                    wm[kb][mb] = t
            onz = const.tile([128, 32], F32R, tag="onesv")
            nc.sync.dma_start(out=onz, in_=onesv[:, :].bitcast(F32R))
            cv = const.tile([128, CV], F32, tag="cv")
            nc.sync.dma_start(out=cv, in_=cvec[0:1, :].partition_broadcast(128))

            def cvs(i):  # per-partition scalar AP for constant i
                return cv[:, i : i + 1]

            # resident x chunks, f32r-typed
            xc = [[None] * N_CHUNK, [None] * N_CHUNK]
            for blk in range(2):
                for c_ in range(N_CHUNK):
                    xc[blk][c_] = xres.tile(
                        [128, CHUNK], F32R,
                        name=f"xc{blk}_{c_}", tag=f"xc{blk}_{c_}",
                    )

            for half in range(2):
                # ---------- phase 1: load + squares + folds + stats ----------
                chR = chp.tile([128, 64], F32, tag=f"chR{half}")
                chS0 = chp.tile([128, 64], F32, tag=f"chS0{half}")
                chS1 = chp.tile([128, 64], F32, tag=f"chS1{half}")

                for t_ in range(N_CHUNK // 2):
                    c_ = half * (N_CHUNK // 2) + t_
                    b_ = (c_ * CHUNK) // HWPIX
                    hw0 = (c_ * CHUNK) % HWPIX
                    for blk in range(2):
                        nc.sync.dma_start(
                            out=xc[blk][c_],
                            in_=xv[blk, :, b_, hw0 : hw0 + CHUNK].bitcast(F32R),
                        )
                    sq0 = sqp.tile([128, CHUNK], F32R, name="sq0", tag="sq0")
                    sq1 = sqp.tile([128, CHUNK], F32, name="sq1", tag="sq1")
                    nc.scalar.activation(
                        sq0, xc[0][c_].bitcast(F32), ACT.Square
                    )
                    nc.scalar.activation(sq1, xc[1][c_].bitcast(F32), ACT.Square)
                    # folds: sqs (in-place into sq0, f32r-rounded) and xs
                    nc.vector.tensor_add(sq0, sq0, sq1)
                    xst = xsp.tile([128, CHUNK], F32R, name="xs", tag="xs")
                    nc.vector.tensor_add(
                        xst, xc[0][c_].bitcast(F32), xc[1][c_].bitcast(F32)
                    )
                    for s4 in range(CHUNK // 512):
                        s = t_ * (CHUNK // 512) + s4  # subtile in half, 0..15
                        sl = slice(s4 * 512, s4 * 512 + 512)
                        st2 = stps.tile([2, 512], F32, tag="st2")
                        nc.tensor.matmul(
                            st2, onz[:, 0:2], xst[:, sl],
                            start=True, stop=True,
                        )
                        st1 = stps.tile([1, 512], F32, tag="st1")
                        nc.tensor.matmul(
                            st1, onz[:, 0:1], sq0[:, sl],
                            start=True, stop=True,
                        )
                        cp = stgp.tile([33, 512], F32, tag="cp")
                        nc.scalar.copy(cp[0:2, :], st2)
                        nc.scalar.copy(cp[32:33, :], st1)
                        nc.sync.dma_start(
                            out=chS0[8 * s : 8 * s + 8, :],
                            in_=cp[0:1, :].rearrange("o (p f) -> o p f", p=8),
                        )
                        nc.sync.dma_start(
                            out=chS1[8 * s : 8 * s + 8, :],
                            in_=cp[1:2, :].rearrange("o (p f) -> o p f", p=8),
                        )
                        nc.sync.dma_start(
                            out=chR[8 * s : 8 * s + 8, :],
                            in_=cp[32:33, :].rearrange("o (p f) -> o p f", p=8),
                        )

                # ---------- phase 2: factored chain on [128, 64] ----------
                r0t = chp.tile([128, 64], F32, tag=f"r0t{half}")
                t0 = chp.tile([128, 64], F32, tag=f"t0{half}")
                t1 = chp.tile([128, 64], F32, tag=f"t1{half}")
                q2t = chp.tile([128, 64], F32, tag=f"q2t{half}")
                tmp = chp.tile([128, 64], F32, tag=f"tmp{half}")
                acc1 = chp.tile([128, 64], F32, tag=f"acc1{half}")
                zt = chp.tile([128, 64], F32, tag=f"zt{half}")
                r2t = chp.tile([128, 64], F32, tag=f"r2t{half}")
                tmp2 = chp.tile([128, 64], F32, tag=f"tmp2{half}")
                acc2 = chp.tile([128, 64], F32, tag=f"acc2{half}")
                At = chp.tile([128, 64], F32, tag=f"At{half}")

                # r0 = clamp(sqrt(R))
                nc.scalar.activation(r0t, chR, ACT.Sqrt)
                nc.vector.tensor_scalar(
                    r0t, r0t, cvs(ID_R0LO), cvs(ID_R0HI), AT.max, AT.min
                )
                # q^2 = R + nd0*S0^2 + nd1*S1^2
                nc.vector.tensor_mul(t0, chS0, chS0)
                nc.vector.scalar_tensor_tensor(
                    q2t, t0, cvs(ID_ND0), chR, AT.mult, AT.add
                )
                nc.vector.tensor_mul(t1, chS1, chS1)
                nc.vector.scalar_tensor_tensor(
                    q2t, t1, cvs(ID_ND1), q2t, AT.mult, AT.add
                )
                # P1(r0) = a1N * prod_j (r0^2 + p_j r0 + q_j); chR == r0^2
                nc.vector.scalar_tensor_tensor(
                    tmp, r0t, cvs(ID_P1P), chR, AT.mult, AT.add
                )
                nc.vector.tensor_scalar(
                    acc1, tmp, cvs(ID_P1Q), cvs(ID_A1N), AT.add, AT.mult
                )
                for j in range(1, NQ):
                    nc.vector.scalar_tensor_tensor(
                        tmp, r0t, cvs(ID_P1P + j), chR, AT.mult, AT.add
                    )
                    nc.vector.scalar_tensor_tensor(
                        acc1, tmp, cvs(ID_P1Q + j), acc1, AT.add, AT.mult
                    )
                # z = P1^2 * q^2; r2 = clamp(sqrt(z))
                nc.vector.tensor_mul(zt, acc1, acc1)
                nc.vector.tensor_mul(zt, zt, q2t)
                nc.scalar.activation(r2t, zt, ACT.Sqrt)
                nc.vector.tensor_scalar(
                    r2t, r2t, cvs(ID_R2LO), cvs(ID_R2HI), AT.max, AT.min
                )
                # Q2(r2) (alpha folded into a2N); zt == r2^2
                nc.vector.scalar_tensor_tensor(
                    tmp2, r2t, cvs(ID_Q2P), zt, AT.mult, AT.add
                )
                nc.vector.tensor_scalar(
                    acc2, tmp2, cvs(ID_Q2Q), cvs(ID_A2N), AT.add, AT.mult
                )
                for j in range(1, NQ):
                    nc.vector.scalar_tensor_tensor(
                        tmp2, r2t, cvs(ID_Q2P + j), zt, AT.mult, AT.add
                    )
                    nc.vector.scalar_tensor_tensor(
                        acc2, tmp2, cvs(ID_Q2Q + j), acc2, AT.add, AT.mult
                    )
                nc.vector.tensor_mul(At, acc1, acc2)

                # ---------- phase 3: w = Wc@x, out = A*w (+ beta*x) ----------
                for g in range(N_HALF_SUB // 2):
                    subs = (2 * g, 2 * g + 1)
                    abt = {}
                    for s in subs:
                        arow = asp.tile([1, 512], F32, tag="arow", name=f"ar{s}")
                        nc.sync.dma_start(
                            out=arow[0:1, :].rearrange("o (p f) -> o p f", p=8),
                            in_=At[8 * s : 8 * s + 8, :],
                        )
                        ab = abp.tile([128, 512], F32, tag="ab", name=f"ab{s}")
                        nc.gpsimd.partition_broadcast(ab, arow[0:1, :])
                        abt[s] = ab
                    wt = {}
                    for mb in range(2):
                        for kb in range(2):
                            for s in subs:
                                gpix = half * HALF + s * 512
                                c_ = gpix // CHUNK
                                off = gpix % CHUNK
                                sl = slice(off, off + 512)
                                if kb == 0:
                                    wt[(mb, s)] = wps.tile(
                                        [128, 512], F32,
                                        tag=f"w{mb}", name=f"w{mb}_{s}",
                                    )
                                nc.tensor.matmul(
                                    wt[(mb, s)], wm[kb][mb],
                                    xc[kb][c_][:, sl],
                                    start=(kb == 0), stop=(kb == 1),
                                )
                    for s in subs:
                        gpix = half * HALF + s * 512
                        c_ = gpix // CHUNK
                        off = gpix % CHUNK
                        sl = slice(off, off + 512)
                        b_ = gpix // HWPIX
                        hw0 = gpix % HWPIX
                        ot = outp.tile([128, 2, 512], F32, tag="ot")
                        for mb in range(2):
                            nc.vector.tensor_mul(
                                ot[:, mb, :], wt[(mb, s)], abt[s]
                            )
                            if beta_nonzero:
                                nc.vector.scalar_tensor_tensor(
                                    ot[:, mb, :],
                                    xc[mb][c_][:, sl].bitcast(F32),
                                    cvs(ID_BETA),
                                    ot[:, mb, :],
                                    AT.mult,
                                    AT.add,
                                )
                        nc.sync.dma_start(
                            out=ov[:, :, b_, hw0 : hw0 + 512].rearrange(
                                "cb p hw -> p cb hw"
                            ),
                            in_=ot,
                        )
    nc.finalize()
    return nc


def _chain_funcs(a0_1, a_1, b_1, a0_2, a_2, b_2):
    sc = np.sqrt(C_CURV)
    n = np.arange(1, N_HARM + 1)

    def fser(r, a0_, a, b):
        return (
            a0_
            + np.cos(np.outer(r, n)) @ np.asarray(a, np.float64)
            + np.sin(np.outer(r, n)) @ np.asarray(b, np.float64)
        )

    def g_of_r(r):
        rn = np.maximum(r, EPS)
        arg = np.minimum(sc * rn, 1 - 1e-5)
        return np.arctanh(arg) / (sc * rn)

    def P1r(r):
        g = g_of_r(r)
        rn1 = np.maximum(np.abs(g) * r, EPS)
        return (g * fser(rn1, a0_1, a_1, b_1).reshape(r.shape))

    def Q2r(r2):
        f2 = fser(r2, a0_2, a_2, b_2).reshape(r2.shape)
        r3 = np.maximum(np.abs(f2) * r2, EPS)
        return f2 * np.tanh(sc * r3) / (sc * r3)

    return P1r, Q2r


def _fit_factored(f, lo, hi, deg):
    """Chebyshev fit of f on [lo,hi], returned as (aN, [(p_j,q_j)]) with
    f(r) ~= aN * prod_j (r^2 + p_j r + q_j)."""
    xs = np.linspace(lo, hi, 6001)
    u = (2 * xs - (lo + hi)) / (hi - lo)
    cc = np.polynomial.chebyshev.chebfit(u, f(xs), deg)
    r_u = np.polynomial.chebyshev.chebroots(cc)
    pc = np.polynomial.chebyshev.cheb2poly(cc)
    aN_u = pc[-1]
    s = 2.0 / (hi - lo)
    t = -(lo + hi) / (hi - lo)
    r_y = (r_u - t) / s
    aN_y = float(aN_u * s**deg)
    cpos = [r for r in r_y if r.imag > 1e-9]
    reals = sorted([r.real for r in r_y if abs(r.imag) <= 1e-9])
    quads = [(-2 * r.real, abs(r) ** 2) for r in cpos]
    for i in range(0, len(reals), 2):
        quads.append((-(reals[i] + reals[i + 1]), reals[i] * reals[i + 1]))
    assert len(quads) == deg // 2, (len(quads), deg)
    return aN_y, quads


def _build_wmat(phi):
    """Wrows[i, j] such that v0_row = u1_row @ Wrows, in float64 then f32."""
    phi = np.asarray(phi, np.float64)
    ang = L * phi
    hf = np.cos(ang) + 1j * np.sin(ang)
    eye = np.eye(C, dtype=np.float64)
    wrows = np.fft.irfft(
        np.fft.rfft(eye, axis=1) * hf[None, : C // 2 + 1], n=C, axis=1
    )
    wm = np.empty((2, 2, 128, 128), np.float32)
    for kb in range(2):
        for mb in range(2):
            wm[kb, mb] = wrows[
                128 * kb : 128 * kb + 128, 128 * mb : 128 * mb + 128
            ].astype(np.float32)
    return wm


_PROGRAM_CACHE: dict = {}


def prepare(inputs):
    """Build (nc, in_maps) for the SPMD run from full inputs."""
    x = np.ascontiguousarray(np.asarray(inputs["x"], dtype=np.float32))
    a0_1 = float(np.asarray(inputs["a0_1"]).reshape(-1)[0])
    a_1 = np.asarray(inputs["a_1"], np.float64)
    b_1 = np.asarray(inputs["b_1"], np.float64)
    a0_2 = float(np.asarray(inputs["a0_2"]).reshape(-1)[0])
    a_2 = np.asarray(inputs["a_2"], np.float64)
    b_2 = np.asarray(inputs["b_2"], np.float64)
    phi = np.asarray(inputs["phi"], np.float64)
    alpha = float(np.asarray(inputs["alpha"]).reshape(-1)[0])
    beta = float(np.asarray(inputs["beta"]).reshape(-1)[0])

    wm = _build_wmat(phi)

    cos0 = np.cos(L * phi[0])
    cos128 = np.cos(L * phi[128])
    nd0 = -(1.0 - cos0 * cos0) / C
    nd1 = -(1.0 - cos128 * cos128) / C

    # exact per-pixel stat ranges from the data (f64)
    xr = x.reshape(B, C, HWPIX).astype(np.float64)
    r0sq = np.einsum("bcp,bcp->bp", xr, xr)
    S0 = xr.sum(axis=1)
    sgn = (1.0 - 2.0 * (np.arange(C) % 2))[None, :, None]
    S1 = (xr * sgn).sum(axis=1)
    qsq = r0sq + nd0 * S0**2 + nd1 * S1**2
    r0 = np.sqrt(r0sq)

    P1r, Q2r = _chain_funcs(a0_1, a_1, b_1, a0_2, a_2, b_2)
    m = 0.003
    r0lo, r0hi = float(r0.min()) * (1 - m), float(r0.max()) * (1 + m)
    p1 = P1r(r0.ravel()).reshape(r0.shape)
    r2 = np.sqrt(np.maximum(p1 * p1 * qsq, 0.0))
    r2lo, r2hi = float(r2.min()) * (1 - m), float(r2.max()) * (1 + m)

    a1N, quads1 = _fit_factored(P1r, r0lo, r0hi, DEG)
    a2N, quads2 = _fit_factored(Q2r, r2lo, r2hi, DEG)

    cvec = np.zeros((1, CV), np.float32)
    for j, (p, q) in enumerate(quads1):
        cvec[0, ID_P1P + j] = p
        cvec[0, ID_P1Q + j] = q
    cvec[0, ID_A1N] = a1N
    for j, (p, q) in enumerate(quads2):
        cvec[0, ID_Q2P + j] = p
        cvec[0, ID_Q2Q + j] = q
    cvec[0, ID_A2N] = a2N * alpha
    cvec[0, ID_ND0] = nd0
    cvec[0, ID_ND1] = nd1
    cvec[0, ID_R0LO] = r0lo
    cvec[0, ID_R0HI] = r0hi
    cvec[0, ID_R2LO] = r2lo
    cvec[0, ID_R2HI] = r2hi
    cvec[0, ID_BETA] = beta

    onesv = np.zeros((128, 32), np.float32)
    onesv[:, 0] = 1.0
    onesv[:, 1] = 1.0 - 2.0 * (np.arange(128) % 2)

    beta_nonzero = beta != 0.0
    key = beta_nonzero
    if key not in _PROGRAM_CACHE:
        _PROGRAM_CACHE[key] = build_program(beta_nonzero)
    nc = _PROGRAM_CACHE[key]

    xr32 = x.reshape(B, C, HWPIX)
    in_maps = []
    for k in range(NCORES):
        in_maps.append(
            {
                "x": xr32[k * B_CORE : (k + 1) * B_CORE],
                "wmat": wm,
                "onesv": onesv,
                "cvec": cvec,
            }
        )
    return nc, in_maps


def kernel(**inputs) -> np.ndarray:
    nc, in_maps = prepare(inputs)

    from concourse.bass_utils import run_bass_kernel_spmd

    res = run_bass_kernel_spmd(nc, in_maps, list(range(NCORES)))
    out = np.concatenate([np.asarray(r["out"]) for r in res.results], axis=0)
    return out.reshape(B, C, H, W)


# revision 13
# speedup vs baseline: 1.1118x; 1.1118x over previous
"""Trainium2 Bass kernel for nn_ChannelNonlinearSpectralBlock.

Math
----
Per pixel column x (C=256 channels), the reference computes
    u  = g(||x||) * x                      (log map, per-pixel scalar gate)
    u1 = f1(||u||) * u                     (Fourier gate 1)
    v0 = irfft(rfft(u1) * Hf)              (fixed linear map: circulant Wc)
    v1 = f2(||v0||) * v0                   (Fourier gate 2)
    y  = t(||v1||) * v1                    (exp map)
    out = alpha*y + beta*x
All per-pixel scalars commute through the linear map Wc, so
    out = A * (Wc @ x) + beta * x
with A = alpha * P1(r0) * Q2(r2), where r0 = ||x||, r2 = |P1| * q,
q^2 = r0^2 - d0*S0^2 - d1*S1^2 (Parseval; S0 = sum_c x_c, S1 = alternating
sum).  P1, Q2 are smooth 1-D functions of r fitted host-side as Chebyshev
approximants and evaluated on-device in factored quadratic form
    P(r) = aN * prod_j (r^2 + p_j r + q_j)
using fused scalar_tensor_tensor ops (2 DVE ops per quadratic factor).
Fit domains are exact data ranges computed host-side from x.

Per chunk of 2048 pixels, channel blocks are folded (xs = x0+x1,
sqs = x0^2+x1^2) before the ones-matmul reduce, halving PE streams; the
three per-pixel stats land in one PSUM tile (partitions 0,1 and 32 via
matmul tile_position) so one Scalar copy drains them, then tiny
SBUF->SBUF DMAs pack the [128, 64] chain layout.

Sharding: pure data parallel over pixels; core k takes images [4k, 4k+4).
"""

import numpy as np

import concourse.bass as bass
import concourse.bacc as bacc
from concourse import library_config
import concourse.mybir as mybir
from concourse.tile import TileContext

F32 = mybir.dt.float32
F32R = mybir.dt.float32r

# Problem shape (hardcoded per contract)
B, C, H, W = 32, 256, 64, 64
HWPIX = H * W  # 4096
NCORES = 8
B_CORE = B // NCORES  # 4 images per core
NPIX = B_CORE * HWPIX  # 16384 pixels per core
HALF = NPIX // 2  # 8192
N_HALF_SUB = HALF // 512  # 16 subtiles of 512 px per half
CHUNK = 2048  # phase-1 chunk (pixels)
N_CHUNK = NPIX // CHUNK  # 8

C_CURV = 0.001
L = 10
N_HARM = 16
EPS = 1e-6

DEG = 16  # Chebyshev degree; even => DEG//2 quadratic factors
NQ = DEG // 2

# cvec layout
ID_P1P = 0
ID_P1Q = ID_P1P + NQ
ID_A1N = ID_P1Q + NQ
ID_Q2P = ID_A1N + 1
ID_Q2Q = ID_Q2P + NQ
ID_A2N = ID_Q2Q + NQ
ID_ND0 = ID_A2N + 1
ID_ND1 = ID_ND0 + 1
ID_R0LO = ID_ND1 + 1
ID_R0HI = ID_R0LO + 1
ID_R2LO = ID_R0HI + 1
ID_R2HI = ID_R2LO + 1
ID_BETA = ID_R2HI + 1
CV = 64
assert ID_BETA < CV


def build_program(beta_nonzero: bool) -> bass.Bass:
    nc = bacc.Bacc(None, target_bir_lowering=False)
    x = nc.declare_dram_parameter("x", [B_CORE, C, HWPIX], F32, isOutput=False)
    wmat = nc.declare_dram_parameter("wmat", [2, 2, 128, 128], F32, isOutput=False)
    onesv = nc.declare_dram_parameter("onesv", [128, 32], F32, isOutput=False)
    cvec = nc.declare_dram_parameter("cvec", [1, CV], F32, isOutput=False)
    out = nc.declare_dram_parameter("out", [B_CORE, C, HWPIX], F32, isOutput=True)

    # channel-block views of DRAM x / out: [cb, 128, b, hw]
    xv = x.rearrange("b (cb p) hw -> cb p b hw", cb=2)
    ov = out.rearrange("b (cb p) hw -> cb p b hw", cb=2)

    AT = mybir.AluOpType
    ACT = mybir.ActivationFunctionType

    with TileContext(nc) as tc:
        with (
            tc.tile_pool(name="const", bufs=1) as const,
            tc.tile_pool(name="xres", bufs=1) as xres,
            tc.tile_pool(name="sq", bufs=1) as sqp,
            tc.tile_pool(name="xsf", bufs=2) as xsp,
            tc.tile_pool(name="stg", bufs=3) as stgp,
            tc.tile_pool(name="chain", bufs=1) as chp,
            tc.tile_pool(name="astage", bufs=4) as asp,
            tc.tile_pool(name="abp", bufs=3) as abp,
            tc.tile_pool(name="outp", bufs=3) as outp,
            tc.tile_pool(name="stps", bufs=2, space="PSUM") as stps,
            tc.tile_pool(name="wps", bufs=2, space="PSUM") as wps,
        ):
            nc.gpsimd.load_library(library_config.mlp)

            # ---- constants ----
            wm = [[None, None], [None, None]]
            for kb in range(2):
                for mb in range(2):
                    t = const.tile(
                        [128, 128], F32R, name=f"wm{kb}{mb}", tag=f"wm{kb}{mb}"
                    )
                    nc.sync.dma_start(out=t, in_=wmat[kb, mb].bitcast(F32R))
                    wm[kb][mb] = t
            onz = const.tile([128, 32], F32R, tag="onesv")
            nc.sync.dma_start(out=onz, in_=onesv[:, :].bitcast(F32R))
            cv = const.tile([128, CV], F32, tag="cv")
            nc.sync.dma_start(out=cv, in_=cvec[0:1, :].partition_broadcast(128))

            def cvs(i):  # per-partition scalar AP for constant i
                return cv[:, i : i + 1]

            # resident x chunks, f32r-typed
            xc = [[None] * N_CHUNK, [None] * N_CHUNK]
            for blk in range(2):
                for c_ in range(N_CHUNK):
                    xc[blk][c_] = xres.tile(
                        [128, CHUNK], F32R,
                        name=f"xc{blk}_{c_}", tag=f"xc{blk}_{c_}",
                    )

            chT = {}
            for half in range(2):
                # ---------- phase 1: load + squares + folds + stats ----------
                chR = chp.tile([128, 64], F32, tag=f"chR{half}")
                chS0 = chp.tile([128, 64], F32, tag=f"chS0{half}")
                chS1 = chp.tile([128, 64], F32, tag=f"chS1{half}")
                chT[half] = (chR, chS0, chS1)

                for t_ in range(N_CHUNK // 2):
                    c_ = half * (N_CHUNK // 2) + t_
                    b_ = (c_ * CHUNK) // HWPIX
                    hw0 = (c_ * CHUNK) % HWPIX
                    for blk in range(2):
                        nc.sync.dma_start(
                            out=xc[blk][c_],
                            in_=xv[blk, :, b_, hw0 : hw0 + CHUNK].bitcast(F32R),
                        )
                    sq0 = sqp.tile([128, CHUNK], F32R, name="sq0", tag="sq0")
                    sq1 = sqp.tile([128, CHUNK], F32, name="sq1", tag="sq1")
                    nc.scalar.activation(
                        sq0, xc[0][c_].bitcast(F32), ACT.Square
                    )
                    nc.scalar.activation(sq1, xc[1][c_].bitcast(F32), ACT.Square)
                    # folds: sqs (in-place into sq0, f32r-rounded) and xs
                    nc.vector.tensor_add(sq0, sq0, sq1)
                    xst = xsp.tile([128, CHUNK], F32R, name="xs", tag="xs")
                    nc.vector.tensor_add(
                        xst, xc[0][c_].bitcast(F32), xc[1][c_].bitcast(F32)
                    )
                    for s4 in range(CHUNK // 512):
                        s = t_ * (CHUNK // 512) + s4  # subtile in half, 0..15
                        sl = slice(s4 * 512, s4 * 512 + 512)
                        st2 = stps.tile([2, 512], F32, tag="st2")
                        nc.tensor.matmul(
                            st2, onz[:, 0:2], xst[:, sl],
                            start=True, stop=True,
                        )
                        st1 = stps.tile([1, 512], F32, tag="st1")
                        nc.tensor.matmul(
                            st1, onz[:, 0:1], sq0[:, sl],
                            start=True, stop=True,
                        )
                        cp = stgp.tile([33, 512], F32, tag="cp")
                        nc.scalar.copy(cp[0:2, :], st2)
                        nc.scalar.copy(cp[32:33, :], st1)
                        nc.sync.dma_start(
                            out=chS0[8 * s : 8 * s + 8, :],
                            in_=cp[0:1, :].rearrange("o (p f) -> o p f", p=8),
                        )
                        nc.sync.dma_start(
                            out=chS1[8 * s : 8 * s + 8, :],
                            in_=cp[1:2, :].rearrange("o (p f) -> o p f", p=8),
                        )
                        nc.sync.dma_start(
                            out=chR[8 * s : 8 * s + 8, :],
                            in_=cp[32:33, :].rearrange("o (p f) -> o p f", p=8),
                        )

            for half in range(2):
                chR, chS0, chS1 = chT[half]
                # ---------- phase 2: factored chain on [128, 64] ----------
                r0t = chp.tile([128, 64], F32, tag=f"r0t{half}")
                t0 = chp.tile([128, 64], F32, tag=f"t0{half}")
                t1 = chp.tile([128, 64], F32, tag=f"t1{half}")
                q2t = chp.tile([128, 64], F32, tag=f"q2t{half}")
                tmp = chp.tile([128, 64], F32, tag=f"tmp{half}")
                acc1 = chp.tile([128, 64], F32, tag=f"acc1{half}")
                zt = chp.tile([128, 64], F32, tag=f"zt{half}")
                r2t = chp.tile([128, 64], F32, tag=f"r2t{half}")
                tmp2 = chp.tile([128, 64], F32, tag=f"tmp2{half}")
                acc2 = chp.tile([128, 64], F32, tag=f"acc2{half}")
                At = chp.tile([128, 64], F32, tag=f"At{half}")

                # r0 = clamp(sqrt(R))
                nc.scalar.activation(r0t, chR, ACT.Sqrt)
                nc.vector.tensor_scalar(
                    r0t, r0t, cvs(ID_R0LO), cvs(ID_R0HI), AT.max, AT.min
                )
                # q^2 = R + nd0*S0^2 + nd1*S1^2
                nc.vector.tensor_mul(t0, chS0, chS0)
                nc.vector.scalar_tensor_tensor(
                    q2t, t0, cvs(ID_ND0), chR, AT.mult, AT.add
                )
                nc.vector.tensor_mul(t1, chS1, chS1)
                nc.vector.scalar_tensor_tensor(
                    q2t, t1, cvs(ID_ND1), q2t, AT.mult, AT.add
                )
                # P1(r0) = a1N * prod_j (r0^2 + p_j r0 + q_j); chR == r0^2
                nc.vector.scalar_tensor_tensor(
                    tmp, r0t, cvs(ID_P1P), chR, AT.mult, AT.add
                )
                nc.vector.tensor_scalar(
                    acc1, tmp, cvs(ID_P1Q), cvs(ID_A1N), AT.add, AT.mult
                )
                for j in range(1, NQ):
                    nc.vector.scalar_tensor_tensor(
                        tmp, r0t, cvs(ID_P1P + j), chR, AT.mult, AT.add
                    )
                    nc.vector.scalar_tensor_tensor(
                        acc1, tmp, cvs(ID_P1Q + j), acc1, AT.add, AT.mult
                    )
                # z = P1^2 * q^2; r2 = clamp(sqrt(z))
                nc.vector.tensor_mul(zt, acc1, acc1)
                nc.vector.tensor_mul(zt, zt, q2t)
                nc.scalar.activation(r2t, zt, ACT.Sqrt)
                nc.vector.tensor_scalar(
                    r2t, r2t, cvs(ID_R2LO), cvs(ID_R2HI), AT.max, AT.min
                )
                # Q2(r2) (alpha folded into a2N); zt == r2^2
                nc.vector.scalar_tensor_tensor(
                    tmp2, r2t, cvs(ID_Q2P), zt, AT.mult, AT.add
                )
                nc.vector.tensor_scalar(
                    acc2, tmp2, cvs(ID_Q2Q), cvs(ID_A2N), AT.add, AT.mult
                )
                for j in range(1, NQ):
                    nc.vector.scalar_tensor_tensor(
                        tmp2, r2t, cvs(ID_Q2P + j), zt, AT.mult, AT.add
                    )
                    nc.vector.scalar_tensor_tensor(
                        acc2, tmp2, cvs(ID_Q2Q + j), acc2, AT.add, AT.mult
                    )
                nc.vector.tensor_mul(At, acc1, acc2)

                # ---------- phase 3: w = Wc@x, out = A*w (+ beta*x) ----------
                for g in range(N_HALF_SUB // 2):
                    subs = (2 * g, 2 * g + 1)
                    abt = {}
                    for s in subs:
                        arow = asp.tile([1, 512], F32, tag="arow", name=f"ar{s}")
                        nc.sync.dma_start(
                            out=arow[0:1, :].rearrange("o (p f) -> o p f", p=8),
                            in_=At[8 * s : 8 * s + 8, :],
                        )
                        ab = abp.tile([128, 512], F32, tag="ab", name=f"ab{s}")
                        nc.gpsimd.partition_broadcast(ab, arow[0:1, :])
                        abt[s] = ab
                    wt = {}
                    for mb in range(2):
                        for kb in range(2):
                            for s in subs:
                                gpix = half * HALF + s * 512
                                c_ = gpix // CHUNK
                                off = gpix % CHUNK
                                sl = slice(off, off + 512)
                                if kb == 0:
                                    wt[(mb, s)] = wps.tile(
                                        [128, 512], F32,
                                        tag=f"w{mb}", name=f"w{mb}_{s}",
                                    )
                                nc.tensor.matmul(
                                    wt[(mb, s)], wm[kb][mb],
                                    xc[kb][c_][:, sl],
                                    start=(kb == 0), stop=(kb == 1),
                                )
                    for s in subs:
                        gpix = half * HALF + s * 512
                        c_ = gpix // CHUNK
                        off = gpix % CHUNK
                        sl = slice(off, off + 512)
                        b_ = gpix // HWPIX
                        hw0 = gpix % HWPIX
                        ot = outp.tile([128, 2, 512], F32, tag="ot")
                        for mb in range(2):
                            nc.vector.tensor_mul(
                                ot[:, mb, :], wt[(mb, s)], abt[s]
                            )
                            if beta_nonzero:
                                nc.vector.scalar_tensor_tensor(
                                    ot[:, mb, :],
                                    xc[mb][c_][:, sl].bitcast(F32),
                                    cvs(ID_BETA),
                                    ot[:, mb, :],
                                    AT.mult,
                                    AT.add,
                                )
                        nc.sync.dma_start(
                            out=ov[:, :, b_, hw0 : hw0 + 512].rearrange(
                                "cb p hw -> p cb hw"
                            ),
                            in_=ot,
                        )
    nc.finalize()
    return nc


def _chain_funcs(a0_1, a_1, b_1, a0_2, a_2, b_2):
    sc = np.sqrt(C_CURV)
    n = np.arange(1, N_HARM + 1)

    def fser(r, a0_, a, b):
        return (
            a0_
            + np.cos(np.outer(r, n)) @ np.asarray(a, np.float64)
            + np.sin(np.outer(r, n)) @ np.asarray(b, np.float64)
        )

    def g_of_r(r):
        rn = np.maximum(r, EPS)
        arg = np.minimum(sc * rn, 1 - 1e-5)
        return np.arctanh(arg) / (sc * rn)

    def P1r(r):
        g = g_of_r(r)
        rn1 = np.maximum(np.abs(g) * r, EPS)
        return (g * fser(rn1, a0_1, a_1, b_1).reshape(r.shape))

    def Q2r(r2):
        f2 = fser(r2, a0_2, a_2, b_2).reshape(r2.shape)
        r3 = np.maximum(np.abs(f2) * r2, EPS)
        return f2 * np.tanh(sc * r3) / (sc * r3)

    return P1r, Q2r


def _fit_factored(f, lo, hi, deg):
    """Chebyshev fit of f on [lo,hi], returned as (aN, [(p_j,q_j)]) with
    f(r) ~= aN * prod_j (r^2 + p_j r + q_j)."""
    xs = np.linspace(lo, hi, 6001)
    u = (2 * xs - (lo + hi)) / (hi - lo)
    cc = np.polynomial.chebyshev.chebfit(u, f(xs), deg)
    r_u = np.polynomial.chebyshev.chebroots(cc)
    pc = np.polynomial.chebyshev.cheb2poly(cc)
    aN_u = pc[-1]
    s = 2.0 / (hi - lo)
    t = -(lo + hi) / (hi - lo)
    r_y = (r_u - t) / s
    aN_y = float(aN_u * s**deg)
    cpos = [r for r in r_y if r.imag > 1e-9]
    reals = sorted([r.real for r in r_y if abs(r.imag) <= 1e-9])
    quads = [(-2 * r.real, abs(r) ** 2) for r in cpos]
    for i in range(0, len(reals), 2):
        quads.append((-(reals[i] + reals[i + 1]), reals[i] * reals[i + 1]))
    assert len(quads) == deg // 2, (len(quads), deg)
    return aN_y, quads


def _build_wmat(phi):
    """Wrows[i, j] such that v0_row = u1_row @ Wrows, in float64 then f32."""
    phi = np.asarray(phi, np.float64)
    ang = L * phi
    hf = np.cos(ang) + 1j * np.sin(ang)
    eye = np.eye(C, dtype=np.float64)
    wrows = np.fft.irfft(
        np.fft.rfft(eye, axis=1) * hf[None, : C // 2 + 1], n=C, axis=1
    )
    wm = np.empty((2, 2, 128, 128), np.float32)
    for kb in range(2):
        for mb in range(2):
            wm[kb, mb] = wrows[
                128 * kb : 128 * kb + 128, 128 * mb : 128 * mb + 128
            ].astype(np.float32)
    return wm


_PROGRAM_CACHE: dict = {}


def prepare(inputs):
    """Build (nc, in_maps) for the SPMD run from full inputs."""
    x = np.ascontiguousarray(np.asarray(inputs["x"], dtype=np.float32))
    a0_1 = float(np.asarray(inputs["a0_1"]).reshape(-1)[0])
    a_1 = np.asarray(inputs["a_1"], np.float64)
    b_1 = np.asarray(inputs["b_1"], np.float64)
    a0_2 = float(np.asarray(inputs["a0_2"]).reshape(-1)[0])
    a_2 = np.asarray(inputs["a_2"], np.float64)
    b_2 = np.asarray(inputs["b_2"], np.float64)
    phi = np.asarray(inputs["phi"], np.float64)
    alpha = float(np.asarray(inputs["alpha"]).reshape(-1)[0])
    beta = float(np.asarray(inputs["beta"]).reshape(-1)[0])

    wm = _build_wmat(phi)

    cos0 = np.cos(L * phi[0])
    cos128 = np.cos(L * phi[128])
    nd0 = -(1.0 - cos0 * cos0) / C
    nd1 = -(1.0 - cos128 * cos128) / C

    # exact per-pixel stat ranges from the data (f64)
    xr = x.reshape(B, C, HWPIX).astype(np.float64)
    r0sq = np.einsum("bcp,bcp->bp", xr, xr)
    S0 = xr.sum(axis=1)
    sgn = (1.0 - 2.0 * (np.arange(C) % 2))[None, :, None]
    S1 = (xr * sgn).sum(axis=1)
    qsq = r0sq + nd0 * S0**2 + nd1 * S1**2
    r0 = np.sqrt(r0sq)

    P1r, Q2r = _chain_funcs(a0_1, a_1, b_1, a0_2, a_2, b_2)
    m = 0.003
    r0lo, r0hi = float(r0.min()) * (1 - m), float(r0.max()) * (1 + m)
    p1 = P1r(r0.ravel()).reshape(r0.shape)
    r2 = np.sqrt(np.maximum(p1 * p1 * qsq, 0.0))
    r2lo, r2hi = float(r2.min()) * (1 - m), float(r2.max()) * (1 + m)

    a1N, quads1 = _fit_factored(P1r, r0lo, r0hi, DEG)
    a2N, quads2 = _fit_factored(Q2r, r2lo, r2hi, DEG)

    cvec = np.zeros((1, CV), np.float32)
    for j, (p, q) in enumerate(quads1):
        cvec[0, ID_P1P + j] = p
        cvec[0, ID_P1Q + j] = q
    cvec[0, ID_A1N] = a1N
    for j, (p, q) in enumerate(quads2):
        cvec[0, ID_Q2P + j] = p
        cvec[0, ID_Q2Q + j] = q
    cvec[0, ID_A2N] = a2N * alpha
    cvec[0, ID_ND0] = nd0
    cvec[0, ID_ND1] = nd1
    cvec[0, ID_R0LO] = r0lo
    cvec[0, ID_R0HI] = r0hi
    cvec[0, ID_R2LO] = r2lo
    cvec[0, ID_R2HI] = r2hi
    cvec[0, ID_BETA] = beta

    onesv = np.zeros((128, 32), np.float32)
    onesv[:, 0] = 1.0
    onesv[:, 1] = 1.0 - 2.0 * (np.arange(128) % 2)

    beta_nonzero = beta != 0.0
    key = beta_nonzero
    if key not in _PROGRAM_CACHE:
        _PROGRAM_CACHE[key] = build_program(beta_nonzero)
    nc = _PROGRAM_CACHE[key]

    xr32 = x.reshape(B, C, HWPIX)
    in_maps = []
    for k in range(NCORES):
        in_maps.append(
            {
                "x": xr32[k * B_CORE : (k + 1) * B_CORE],
                "wmat": wm,
                "onesv": onesv,
                "cvec": cvec,
            }
        )
    return nc, in_maps


def kernel(**inputs) -> np.ndarray:
    nc, in_maps = prepare(inputs)

    from concourse.bass_utils import run_bass_kernel_spmd

    res = run_bass_kernel_spmd(nc, in_maps, list(range(NCORES)))
    out = np.concatenate([np.asarray(r["out"]) for r in res.results], axis=0)
    return out.reshape(B, C, H, W)
